# revision 53
# baseline (speedup 1.0000x reference)
"""ChildSumTreeLSTM with relation transforms on 8 Trainium2 NeuronCores.

Layout: everything transposed (features on SBUF partitions, tree nodes on the
free dim), node columns in topological-wave order (= heap order for the
reference tree). Per wave of parents:
  hsum (DVE strided reduce over child cols) -> per-slot 0/1 column masks
  (input data) zero the wrong-rel columns of hsum -> rel-sharded PE passes of
  SBUF-cached fp8 (x16-scaled) weight blocks, all slots accumulating into one
  PSUM group -> contrib copied out with a 1/16 activation scale -> bf16
  AllReduce (disjoint supports, exact) gives every core the full ch_sum ->
  column-sharded iou/f gates (each core owns one 128-feature slice); the fh
  matmuls run on PE while the AllReduce is in flight -> small AllGather of the
  new h columns.  xi/xf are folded into the PSUM accumulations with an
  identity-weight matmul so the gate nonlinearities read PSUM directly (fewer
  cross-engine hops).  Waves whose full rel set is cheap to compute everywhere
  are replicated on all cores and skip the AllReduce.
All per-core differences are input data (weight shards, masks, bias slices),
so one Bass program runs SPMD on all 8 cores.  All rel weights are preloaded
into SBUF at kernel start; host-side pre-transposed layouts keep every big
DMA one descriptor per partition.
"""

import sys

sys.path.insert(0, "/opt/trn_rl_repo")

import numpy as np
import ml_dtypes

import concourse.bass as bass
import concourse.mybir as mybir
import concourse.tile as tile
from concourse.bass_utils import run_bass_kernel_spmd
from concourse.vector_clock import ScopedClock, VectorClock

BF16 = mybir.dt.bfloat16
FP8 = mybir.dt.float8e4
F32 = mybir.dt.float32
NP_BF16 = ml_dtypes.bfloat16
NP_FP8 = ml_dtypes.float8_e4m3
NCORES = 8
P = 128
WSCALE = 16.0  # rel weights stored as fp8(W * WSCALE); contrib scaled back

# This walrus build rejects >1 sem wait per instruction at the Tile exit
# drain; split the aggregated drain into one drain per proc.
def _split_drain_and_barrier(self, tick_clock, wait_clock):
    gc = tick_clock.global_clock
    n = len(gc)
    nonzero = [i for i in range(n) if gc[i] > 0]
    for j in nonzero:
        vec = VectorClock([gc[i] if i == j else 0 for i in range(n)])
        d = self.nc.sync.drain()
        wait_clock.add_sem_waits(d.ins, ScopedClock({None: vec}))
    if not nonzero:
        d = self.nc.sync.drain()
        wait_clock.add_sem_waits(d.ins, ScopedClock({None: gc.copy()}))
    self.nc.all_engine_barrier()
    assert self.sems is not None
    popped = self.nc._tile_sem_poison_stack.pop()
    assert popped is self._sem_poison
    self.nc.clear_and_free_semaphores(list(self.sems.allocated().values()))
    self.nc.all_engine_barrier()


tile.TileContext._drain_and_barrier = _split_drain_and_barrier


def _split_multi_waits(nc, limit=1):
    """Walrus here allows only one sem wait per instruction; hoist extras
    onto same-engine NOPs inserted right before the instruction."""
    for bb in nc.main_func.blocks:
        new_list = []
        for ins in bb.instructions:
            si = getattr(ins, "sync_info", None)
            if si is not None and si.on_wait and len(si.on_wait) > limit:
                waits = list(si.on_wait)
                for w in waits[:-limit]:
                    nop = mybir.InstNoOp(
                        name=nc.get_next_instruction_name(),
                        sync_info=mybir.SyncInfo(on_wait=[w], on_update=[]),
                        bass_nofuse=True,
                        engine=ins.engine,
                    )
                    nc.register_instruction(nop, overwrite=True)
                    new_list.append(nop)
                si.on_wait = waits[-limit:]
            new_list.append(ins)
        bb.instructions[:] = new_list


def _bf16(a):
    return np.ascontiguousarray(a.astype(NP_BF16))


def _wslot(mat):
    """[M, K] f32 -> [P, MC*KC, P] fp8 packed lhsT blocks:
    [p, m*KC+k, :] = (mat[mb, kb].T)[p] * WSCALE."""
    M, K = mat.shape
    MC, KC = M // P, K // P
    out = np.empty((P, MC * KC, P), NP_FP8)
    q = (mat.astype(np.float32) * WSCALE).astype(NP_FP8)
    for m in range(MC):
        for k in range(KC):
            out[:, m * KC + k, :] = q[m * P:(m + 1) * P, k * P:(k + 1) * P].T
    return out


def _plan(child_idx, rel_ids, Wrel):
    """Host-side planning: waves, column order, rel->core assignment, slots."""
    N, K = child_idx.shape
    eff_children = []
    wave = np.zeros(N, np.int32)
    for i in range(N):
        cs = [int(c) for c in child_idx[i] if 0 <= c < i]
        eff_children.append(cs)
        wave[i] = 1 + max((wave[c] for c in cs), default=-1)
    nwaves = int(wave.max()) + 1
    # column order: by (wave, descending node) -> for the reference heap tree
    # this is exactly heap order (col j = node N-1-j) keeping children of
    # consecutive parents contiguous.
    order = sorted(range(N), key=lambda i: (wave[i], -i))
    col_of = np.empty(N, np.int64)
    for j, node in enumerate(order):
        col_of[node] = j
    waves = []  # list of (p0, p1) col ranges
    j = 0
    for w in range(nwaves):
        cnt = int((wave == w).sum())
        waves.append((j, j + cnt))
        j += cnt

    ident = set()
    eye = np.eye(Wrel.shape[1], dtype=Wrel.dtype)
    for r in set(int(rel_ids[i]) for i in range(N)):
        if np.array_equal(Wrel[r], eye):
            ident.add(r)

    # per wave (>=1): rels present; identity rels are skipped only when the
    # whole wave is identity (then ch_sum == hsum, no matmul or exchange)
    wave_rels = []
    for w in range(1, nwaves):
        p0, p1 = waves[w]
        rels_all = set(int(rel_ids[order[j]]) for j in range(p0, p1))
        if rels_all <= ident:
            wave_rels.append([])
        else:
            wave_rels.append(sorted(rels_all))

    # static rel->core assignment, greedy balance on total appearances
    from collections import defaultdict
    count = defaultdict(int)
    for rels in wave_rels:
        for r in rels:
            count[r] += 1
    nw = len(wave_rels)
    loadw = [[0] * nw for _ in range(NCORES)]
    assign = {}
    for r in sorted(count, key=lambda r: -count[r]):
        pres = [wi for wi in range(nw) if r in wave_rels[wi]]
        best, bkey = 0, None
        for c in range(NCORES):
            key = (sum(loadw[c][wi] for wi in pres), sum(loadw[c]))
            if bkey is None or key < bkey:
                best, bkey = c, key
        assign[r] = best
        for wi in pres:
            loadw[best][wi] += 1

    # per wave: per-core slot lists, padded to n_s.  Waves whose extra
    # replication cost (every core computing every rel) is below the
    # AllReduce floor are replicated: each core then holds the full ch_sum
    # locally and the wave needs no collective exchange.
    wave_slots = []  # per internal wave: (n_s, slots[c], replicated)
    for rels in wave_rels:
        per_core = [[r for r in rels if assign[r] == c] for c in range(NCORES)]
        n_s = max((len(x) for x in per_core), default=0)
        replicated = 0 < len(rels) and (len(rels) - n_s) <= 3
        if replicated:
            per_core = [list(rels) for _ in range(NCORES)]
            n_s = len(rels)
        wave_slots.append((n_s, per_core, replicated))
    return dict(order=order, col_of=col_of, waves=waves, wave=wave,
                eff_children=eff_children, ident=ident,
                wave_slots=wave_slots, nwaves=nwaves)


def _build(inputs):
    x = np.asarray(inputs["x"], np.float32)
    Wrel = np.asarray(inputs["Wrel"], np.float32)
    ioux_w = np.asarray(inputs["ioux_w"], np.float32)
    ioux_b = np.asarray(inputs["ioux_b"], np.float32)
    iouh_w = np.asarray(inputs["iouh_w"], np.float32)
    iouh_b = np.asarray(inputs["iouh_b"], np.float32)
    fx_w = np.asarray(inputs["fx_w"], np.float32)
    fx_b = np.asarray(inputs["fx_b"], np.float32)
    fh_w = np.asarray(inputs["fh_w"], np.float32)
    fh_b = np.asarray(inputs["fh_b"], np.float32)
    child_idx = np.asarray(inputs["child_idx"], np.int32)
    rel_ids = np.asarray(inputs["rel_ids"], np.int32)

    N, IN_DIM = x.shape
    MEM = fh_w.shape[0]
    KC = MEM // P           # 8 feature chunks
    KX = IN_DIM // P        # 8 input chunks
    K = child_idx.shape[1]  # max children (4)
    NPAD = N + K + 4

    plan = _plan(child_idx, rel_ids, Wrel)
    order, col_of, waves = plan["order"], plan["col_of"], plan["waves"]
    eff_children, ident = plan["eff_children"], plan["ident"]
    wave_slots, nwaves = plan["wave_slots"], plan["nwaves"]

    # Child gather plan: for each internal wave, the flattened (parent-major)
    # child column sequence, decomposed into maximal +1-contiguous runs.
    # Missing children point at the zero pad columns starting at ZCOL.
    ZCOL = N
    child_col = np.full((N, K), -1, np.int64)
    for i in range(N):
        for kk, c in enumerate(eff_children[i]):
            child_col[i, kk] = col_of[c]
    wave_runs = []  # per internal wave: list of (dst_off, src_col, length)
    for w in range(1, nwaves):
        p0, p1 = waves[w]
        seq = []
        for j in range(p0, p1):
            for kk in range(K):
                c = child_col[order[j], kk]
                seq.append(int(c) if c >= 0 else ZCOL + kk)
        runs = []
        i0 = 0
        while i0 < len(seq):
            i1 = i0 + 1
            while i1 < len(seq) and seq[i1] == seq[i1 - 1] + 1:
                i1 += 1
            runs.append((i0, int(seq[i0]), i1 - i0))
            i0 = i1
        wave_runs.append(runs)

    # internal-wave column offsets for the xf replication buffer
    itot = 0
    woff = []  # per internal wave: offset into xfb_all (units of K cols)
    for w in range(1, nwaves):
        woff.append(itot)
        itot += waves[w][1] - waves[w][0]

    # ---- per-core host data -------------------------------------------------
    # all big tensors pre-arranged partition-first so each DMA is one
    # contiguous run per partition
    xT = np.ascontiguousarray(x[order].T)  # [IN_DIM, N] in column order
    xt_h = np.zeros((P, KX, N), NP_BF16)
    for k in range(KX):
        xt_h[:, k, :] = _bf16(xT[k * P:(k + 1) * P])

    S_total = sum(ns for ns, _, _ in wave_slots)
    MC = MEM // P
    S_alloc = max(S_total, 1)
    # partition-major flat layout: [P, S*64*128] so each slice DMA is one
    # contiguous run per partition (cheap descriptor generation)
    wslots = [np.zeros((P, S_alloc, MC * KC, P), NP_FP8) for _ in range(NCORES)]
    NMAX = max((waves[w][1] - waves[w][0]) for w in range(1, nwaves)) if nwaves > 1 else 1
    NBIG = max(p1 - p0 for p0, p1 in waves)
    PSN = 128  # psum column pad so each m-chunk slice stays inside one bank
    assert NMAX <= PSN and K * NMAX <= 512
    masks = [np.zeros((S_alloc, KC, NMAX), NP_BF16) for _ in range(NCORES)]
    soff = 0
    for wi, (ns, per_core, _rep) in enumerate(wave_slots):
        w = wi + 1
        p0, p1 = waves[w]
        n = p1 - p0
        for c in range(NCORES):
            for s, r in enumerate(per_core[c]):
                wslots[c][:, soff + s] = _wslot(Wrel[r])
                for t in range(n):
                    if int(rel_ids[order[p0 + t]]) == r:
                        masks[c][soff + s, :, t] = 1.0
        soff += ns

    iouxstat = [np.zeros((P, KX * 3, P), NP_BF16) for _ in range(NCORES)]
    iouhstat = [np.zeros((P, KC * 3, P), NP_BF16) for _ in range(NCORES)]
    fxstat = [np.zeros((P, KX, P), NP_BF16) for _ in range(NCORES)]
    fhstat = [np.zeros((P, KC, P), NP_BF16) for _ in range(NCORES)]
    b_xi = [np.zeros((3, P), np.float32) for _ in range(NCORES)]
    b_iou = [np.zeros((3, P), np.float32) for _ in range(NCORES)]
    b_xf = [np.zeros((P,), np.float32) for _ in range(NCORES)]
    b_fh = [np.zeros((P,), np.float32) for _ in range(NCORES)]
    for c in range(NCORES):
        rows = slice(c * P, (c + 1) * P)
        for g in range(3):
            gr = slice(g * MEM + c * P, g * MEM + (c + 1) * P)
            b_xi[c][g] = ioux_b[gr]
            b_iou[c][g] = iouh_b[gr]
            for k in range(KX):
                iouxstat[c][:, k * 3 + g, :] = _bf16(
                    ioux_w[gr, k * P:(k + 1) * P].T)
            for k in range(KC):
                iouhstat[c][:, k * 3 + g, :] = _bf16(
                    iouh_w[gr, k * P:(k + 1) * P].T)
        b_xf[c] = fx_b[rows]
        b_fh[c] = fh_b[rows]
        for k in range(KX):
            fxstat[c][:, k, :] = _bf16(fx_w[rows, k * P:(k + 1) * P].T)
        for k in range(KC):
            fhstat[c][:, k, :] = _bf16(fh_w[rows, k * P:(k + 1) * P].T)
    eye_h = _bf16(np.eye(P, dtype=np.float32))

    # ---- build program ------------------------------------------------------
    nc = bass.Bass("TRN2", target_bir_lowering=False, debug=False,
                   num_devices=NCORES)
    d_ws = nc.dram_tensor("wslots", [P, S_alloc, MC * KC, P], FP8,
                          kind="ExternalInput")
    masks_x = [np.ascontiguousarray(
        np.broadcast_to(m[None], (P,) + m.shape)) for m in masks]
    d_mask = nc.dram_tensor("masks", list(masks_x[0].shape), BF16,
                            kind="ExternalInput")
    d_xt = nc.dram_tensor("xt", [P, KX, N], BF16, kind="ExternalInput")
    d_iouxs = nc.dram_tensor("iouxstat", [P, KX * 3, P], BF16, kind="ExternalInput")
    d_iouhs = nc.dram_tensor("iouhstat", [P, KC * 3, P], BF16, kind="ExternalInput")
    d_fxs = nc.dram_tensor("fxstat", [P, KX, P], BF16, kind="ExternalInput")
    d_fhs = nc.dram_tensor("fhstat", [P, KC, P], BF16, kind="ExternalInput")
    d_eye = nc.dram_tensor("eye", [P, P], BF16, kind="ExternalInput")
    d_bxi = nc.dram_tensor("b_xi", [3, P], F32, kind="ExternalInput")
    d_biou = nc.dram_tensor("b_iou", [3, P], F32, kind="ExternalInput")
    d_bxf = nc.dram_tensor("b_xf", [P], F32, kind="ExternalInput")
    d_bfh = nc.dram_tensor("b_fh", [P], F32, kind="ExternalInput")
    d_hout = nc.dram_tensor("hout", [P, N], BF16, kind="ExternalOutput")

    with tile.TileContext(nc, num_cores=NCORES) as tc:
        with (
            tc.tile_pool(name="const", bufs=1) as cpool,
            tc.tile_pool(name="state", bufs=1) as spool,
            tc.tile_pool(name="work", bufs=1) as wk,
            tc.tile_pool(name="mselp", bufs=1) as mselp,
            tc.tile_pool(name="psmm", bufs=1, space="PSUM") as pp,
            tc.tile_pool(name="psg", bufs=2, space="PSUM") as pg,
            tc.tile_pool(name="dram", bufs=2, space="DRAM") as dp,
        ):
            # constants needed for the leaf phase first
            xt = cpool.tile([P, KX, N], BF16)
            nc.sync.dma_start(xt[:], d_xt.ap())
            iouxs = cpool.tile([P, KX * 3, P], BF16)
            nc.sync.dma_start(iouxs[:], d_iouxs.ap())
            fxs = cpool.tile([P, KX, P], BF16)
            nc.sync.dma_start(fxs[:], d_fxs.ap())
            eye = cpool.tile([P, P], BF16)
            nc.sync.dma_start(eye[:], d_eye.ap())
            bxi = cpool.tile([P, 3], F32)
            nc.sync.dma_start(bxi[:], d_bxi.ap().rearrange("g p -> p g"))
            biou = cpool.tile([P, 3], F32)
            nc.sync.dma_start(biou[:], d_biou.ap().rearrange("g p -> p g"))
            bxf = cpool.tile([P, 1], F32)
            nc.sync.dma_start(bxf[:], d_bxf.ap().rearrange("(p one) -> p one", one=1))
            bfh = cpool.tile([P, 1], F32)
            nc.sync.dma_start(bfh[:], d_bfh.ap().rearrange("(p one) -> p one", one=1))
            # combined xi+iou bias for the leaf gates (they read the raw
            # ioux-matmul psum directly)
            bxiou = cpool.tile([P, 3], F32)
            nc.vector.tensor_add(bxiou[:], bxi[:], biou[:])

            # bulk prefetch tiles (DMAs issued later, on the Activation
            # HW-DGE ring, so the SP ring stays clean for latency-critical
            # staging around the collectives)
            iouhs = cpool.tile([P, KC * 3, P], BF16)
            fhs = cpool.tile([P, KC, P], BF16)
            msk = cpool.tile([P, S_alloc, KC, NMAX], BF16)
            wrel = cpool.tile([P, S_alloc * MC * KC, P], FP8)

            # state
            h_bf = spool.tile([P, KC, NPAD], BF16)
            nc.vector.memset(h_bf[:], 0.0)
            c_sl = spool.tile([P, NPAD], F32)
            nc.vector.memset(c_sl[:], 0.0)
            h_sl = spool.tile([P, N], BF16)
            xi_bf = spool.tile([P, 3, N], BF16)
            xf_bf = spool.tile([P, N], BF16)
            xfb_all = spool.tile([P, K * max(itot, 1)], BF16)

            ACT = mybir.ActivationFunctionType

            def gates(p0, n, psi, psf=None, cc_pieces=None,
                      nch=0, bias_t=None):
                """Column-sharded gate math for parents at cols [p0, p0+n).
                psi: [P,3,n] PSUM accumulation including xi (via eye matmul,
                or raw ioux psum for leaves with bias_t=bxiou).
                psf: [P,nch] PSUM fh+xf accumulation, or None for leaves.
                cc_main: direct c_sl AP for the contiguous child block,
                cc_tail: staged c for the remaining child cols."""
                bt = biou if bias_t is None else bias_t
                ig = wk.tile([P, NBIG], F32, tag="ig")
                og = wk.tile([P, NBIG], F32, tag="og")
                ug = wk.tile([P, NBIG], F32, tag="ug")
                nc.scalar.activation(ig[:, :n], psi[:, 0, :n], ACT.Sigmoid,
                                     bias=bt[:, 0:1])
                nc.scalar.activation(og[:, :n], psi[:, 1, :n], ACT.Sigmoid,
                                     bias=bt[:, 1:2])
                nc.scalar.activation(ug[:, :n], psi[:, 2, :n], ACT.Tanh,
                                     bias=bt[:, 2:3])
                if psf is not None:
                    fsb = wk.tile([P, K * NMAX], F32, tag="fsb")
                    nc.scalar.activation(fsb[:, :nch], psf, ACT.Sigmoid,
                                         bias=bfh[:, 0:1])
                cn = wk.tile([P, NBIG], F32, tag="cn")
                nc.vector.tensor_mul(cn[:, :n], ig[:, :n], ug[:, :n])
                if psf is not None:
                    for lo, hi, ap in cc_pieces:
                        nc.vector.tensor_mul(fsb[:, lo:hi],
                                             fsb[:, lo:hi], ap)
                    fc = wk.tile([P, NMAX], F32, tag="fc")
                    nc.vector.tensor_reduce(
                        fc[:, :n],
                        fsb[:, :nch].rearrange("p (n k) -> p n k", k=K),
                        axis=mybir.AxisListType.X, op=mybir.AluOpType.add)
                    nc.vector.tensor_add(c_sl[:, p0:p0 + n], cn[:, :n],
                                         fc[:, :n])
                else:
                    nc.vector.tensor_copy(c_sl[:, p0:p0 + n], cn[:, :n])
                tct = wk.tile([P, NBIG], F32, tag="tct")
                nc.scalar.activation(tct[:, :n], c_sl[:, p0:p0 + n], ACT.Tanh)
                with nc.allow_low_precision(reason="h is published in bf16"):
                    nc.vector.tensor_mul(h_sl[:, p0:p0 + n], og[:, :n],
                                         tct[:, :n])

            def publish_h(p0, n):
                # staging DMAs ride the Pool queue (SWDGE) right before the
                # trigger: in-order issue and a much cheaper completion sem
                sfx = str(n)
                gin = dp.tile([P, n], BF16, tag="gin" + sfx)
                nc.gpsimd.dma_start(gin[:], h_sl[:, p0:p0 + n])
                gout = dp.tile([NCORES, P, n], BF16, tag="gout" + sfx,
                               addr_space="Shared")
                nc.gpsimd.collective_compute(
                    "AllGather", mybir.AluOpType.bypass,
                    ins=[gin.opt()], outs=[gout.opt()],
                    replica_groups=[list(range(NCORES))])
                nc.gpsimd.dma_start(
                    h_bf[:, :, p0:p0 + n],
                    gout[:, :, :n].rearrange("k p n -> p k n"))

            # ---- wave 0 fused with the xi/xf precompute: leaf chunks gate
            # straight off the ioux psum; internal chunks store xi/xf
            p0, p1 = waves[0]
            n0 = p1 - p0
            CCH = PSN
            for cc in range(0, N, CCH):
                ncc = min(CCH, N - cc)
                ps = pg.tile([P, 3, PSN], F32, tag="ps3")
                for g in range(3):
                    for k in range(KX):
                        nc.tensor.matmul(
                            ps[:, g, :ncc],
                            iouxs[:, k * 3 + g, :],
                            xt[:, k, cc:cc + ncc],
                            start=(k == 0), stop=(k == KX - 1))
                nl = max(0, min(ncc, n0 - cc))        # leading leaf cols
                if nl:
                    gates(cc, nl, ps, bias_t=bxiou)
                    nc.sync.dma_start(d_hout.ap()[:, cc:cc + nl],
                                      h_sl[:, cc:cc + nl])
                if nl < ncc:                          # internal cols
                    o = nl
                    for g in range(3):
                        nc.scalar.activation(
                            xi_bf[:, g, cc + o:cc + ncc], ps[:, g, o:ncc],
                            ACT.Identity, bias=bxi[:, g:g + 1])
                    psf0 = pg.tile([P, K * NMAX], F32, tag="psf")
                    for k in range(KX):
                        nc.tensor.matmul(
                            psf0[:, o:ncc], fxs[:, k, :],
                            xt[:, k, cc + o:cc + ncc],
                            start=(k == 0), stop=(k == KX - 1))
                    nc.scalar.activation(
                        xf_bf[:, cc + o:cc + ncc], psf0[:, o:ncc],
                        ACT.Identity, bias=bxf[:, 0:1])
            publish_h(p0, n0)

            # xf replicated 4x per child slot for every internal wave
            # (feeds the f-gate psum via an identity matmul); off critical path
            for w in range(1, nwaves):
                pw0, pw1 = waves[w]
                nw = pw1 - pw0
                off = woff[w - 1] * K
                v = xfb_all[:, off:off + nw * K].rearrange(
                    "p (n k) -> p n k", k=K)
                for kk in range(K):
                    nc.vector.tensor_copy(
                        v[:, :, kk:kk + 1],
                        xf_bf[:, pw0:pw0 + nw].rearrange(
                            "p (n one) -> p n one", one=1))

            # bulk prefetch on the Activation HW-DGE ring, emitted after the
            # leaf-phase Act work so it doesn't delay the leaf gates; masks
            # are split per wave so wave 1's slice lands first
            soff_d = 0
            for wi in range(1, nwaves):
                ns_w = wave_slots[wi - 1][0]
                if ns_w:
                    nc.scalar.dma_start(
                        msk[:, soff_d:soff_d + ns_w], d_mask.ap()[:, soff_d:soff_d + ns_w])
                soff_d += ns_w
            nc.scalar.dma_start(iouhs[:], d_iouhs.ap())
            nc.scalar.dma_start(fhs[:], d_fhs.ap())
            soff_d = 0
            for wi in range(1, nwaves):
                ns_w = wave_slots[wi - 1][0]
                if ns_w:
                    nc.scalar.dma_start(
                        wrel[:, soff_d * MC * KC:(soff_d + ns_w) * MC * KC, :],
                        d_ws.ap()[:, soff_d:soff_d + ns_w])
                soff_d += ns_w

            # ---- internal waves -------------------------------------------
            soff = 0
            for wi in range(1, nwaves):
                ns, per_core, rep = wave_slots[wi - 1]
                p0, p1 = waves[wi]
                n = p1 - p0
                nch = n * K
                # the heap column order makes all real children one
                # contiguous block in h_bf/c_sl (only the last parent may be
                # partial), and the ZCOL pad area is zeros — so everything is
                # read via direct APs, no staging copies at all
                runs = wave_runs[wi - 1]
                real = [r for r in runs if r[1] < ZCOL]
                assert real and real[0][0] == 0
                src0 = real[0][1]
                tot = sum(r[2] for r in real)   # real child cols
                assert real[-1][0] + real[-1][2] == tot  # contiguous dst
                main_np = real[0][2] // K
                main_ln = main_np * K
                rem = real[0][2] - main_ln      # partial parent in run 0
                # pieces beyond the main block all belong to the last parent
                assert (not real[1:] and not rem) or main_np >= n - 1
                assert tot - main_ln <= K
                # hsum over child cols (bf16 in, bf16 out); one 4D reduce
                # covers all feature chunks, the last parent's extra run
                # pieces join via reduce+add
                hsum_b = wk.tile([P, KC, NMAX], BF16, tag="hsumb")
                lastp = n - 1
                with nc.allow_low_precision(reason="4-term bf16 child sum"):
                    if main_np:
                        nc.vector.tensor_reduce(
                            hsum_b[:, :, :main_np],
                            h_bf[:, :, src0:src0 + main_ln].rearrange(
                                "p k (n c) -> p k n c", c=K),
                            axis=mybir.AxisListType.X,
                            op=mybir.AluOpType.add)
                    init = False
                    if rem:
                        nc.vector.tensor_reduce(
                            hsum_b[:, :, lastp:lastp + 1],
                            h_bf[:, :, src0 + main_ln:src0 + real[0][2]]
                            .rearrange("p k (n c) -> p k n c", c=rem),
                            axis=mybir.AxisListType.X,
                            op=mybir.AluOpType.add)
                        init = True
                    hs2 = wk.tile([P, KC, 1], BF16, tag="hs2")
                    for (dst, src, ln) in real[1:]:
                        dstt = hsum_b if not init else hs2
                        nc.vector.tensor_reduce(
                            dstt[:, :, lastp:lastp + 1] if not init
                            else hs2[:, :, 0:1],
                            h_bf[:, :, src:src + ln].rearrange(
                                "p k (n c) -> p k n c", c=ln),
                            axis=mybir.AxisListType.X,
                            op=mybir.AluOpType.add)
                        if init:
                            nc.vector.tensor_add(
                                hsum_b[:, :, lastp:lastp + 1],
                                hsum_b[:, :, lastp:lastp + 1],
                                hs2[:, :, 0:1])
                        init = True

                all_id = (ns == 0)
                # replicated single-rel wave: every mask is all-ones, feed
                # hsum straight to the matmul
                uniform = rep and ns == 1
                if not all_id:
                    psl = pp.tile([P, MC, PSN], F32, tag="psl")
                    msels = []
                    for s in range(ns):
                        if uniform:
                            msels.append(hsum_b)
                            continue
                        msel = mselp.tile([P, KC, NMAX], BF16,
                                          tag="msel" + str(s))
                        eng = nc.vector if s % 2 == 0 else nc.gpsimd
                        eng.tensor_mul(msel[:, :, :n], hsum_b[:, :, :n],
                                       msk[:, soff + s, :, :n])
                        msels.append(msel)
                    # m-outer so each psum region's accumulation group
                    # (spanning all slots) closes before the next opens
                    for m in range(MC):
                        for s in range(ns):
                            for k in range(KC):
                                nc.tensor.matmul(
                                    psl[:, m, :n],
                                    wrel[:, (soff + s) * MC * KC + m * KC + k, :],
                                    msels[s][:, k, :n],
                                    start=(s == 0 and k == 0),
                                    stop=(s == ns - 1 and k == KC - 1))
                    # scale fp8 weights back (1/WSCALE) on the psum read
                    cb = wk.tile([P, KC, n], BF16, tag="cb" + str(n))
                    nc.scalar.activation(
                        cb[:, :, :n], psl[:, :, :n], ACT.Identity,
                        scale=1.0 / WSCALE)
                    if rep:
                        rhs = cb     # every core computed the full ch_sum
                    else:
                        g1in = dp.tile([P, KC, n], BF16, tag="g1in" + str(n))
                        nc.gpsimd.dma_start(g1in[:], cb[:])
                        g1out = dp.tile([P, KC, n], BF16, tag="g1out" + str(n),
                                        addr_space="Shared")
                        # contributions have disjoint support (masked), so
                        # the bf16 CCE adds are exact
                        nc.gpsimd.collective_compute(
                            "AllReduce", mybir.AluOpType.add,
                            ins=[g1in.opt()], outs=[g1out.opt()],
                            replica_groups=[list(range(NCORES))])
                        chs_b = wk.tile([P, KC, n], BF16, tag="chsb" + str(n))
                        nc.gpsimd.dma_start(chs_b[:], g1out[:])
                        rhs = chs_b
                else:
                    rhs = hsum_b

                # fh matmuls first: they read h_bf directly, so PE runs
                # them while the AllReduce is in flight; xf joins via eye mm.
                # Cols past the real children only get the xf term (their
                # c factor is zero anyway).
                psf = pg.tile([P, K * NMAX], F32, tag="psf")
                off = woff[wi - 1] * K
                for (dst, src, ln) in real:
                    for k in range(KC):
                        nc.tensor.matmul(
                            psf[:, dst:dst + ln], fhs[:, k, :],
                            h_bf[:, k, src:src + ln],
                            start=(k == 0), stop=False)
                    nc.tensor.matmul(psf[:, dst:dst + ln], eye[:],
                                     xfb_all[:, off + dst:off + dst + ln],
                                     start=False, stop=True)
                if nch > tot:
                    nc.tensor.matmul(psf[:, tot:nch], eye[:],
                                     xfb_all[:, off + tot:off + nch],
                                     start=True, stop=True)
                # iou matmuls (column-sharded); xi joins via eye matmul
                psi = pg.tile([P, 3, PSN], F32, tag="ps3")
                for g in range(3):
                    for k in range(KC):
                        nc.tensor.matmul(
                            psi[:, g, :n], iouhs[:, k * 3 + g, :],
                            rhs[:, k, :n],
                            start=(k == 0), stop=False)
                    nc.tensor.matmul(psi[:, g, :n], eye[:],
                                     xi_bf[:, g, p0:p0 + n],
                                     start=False, stop=True)
                pieces = [(dst, dst + ln, c_sl[:, src:src + ln])
                          for (dst, src, ln) in real]
                if nch > tot:
                    pieces.append((tot, nch,
                                   c_sl[:, ZCOL:ZCOL + (nch - tot)]))
                gates(p0, n, psi, psf[:, :nch], cc_pieces=pieces, nch=nch)
                nc.sync.dma_start(d_hout.ap()[:, p0:p0 + n],
                                  h_sl[:, p0:p0 + n])
                if wi < nwaves - 1:
                    publish_h(p0, n)
                soff += ns

    in_maps = []
    for c in range(NCORES):
        in_maps.append({
            "wslots": wslots[c], "masks": masks_x[c],
            "xt": xt_h, "iouxstat": iouxstat[c], "iouhstat": iouhstat[c],
            "fxstat": fxstat[c], "fhstat": fhstat[c], "eye": eye_h,
            "b_xi": b_xi[c], "b_iou": b_iou[c], "b_xf": b_xf[c],
            "b_fh": b_fh[c],
        })
    _split_multi_waits(nc)
    return nc, in_maps, col_of, N, MEM


def kernel(**inputs):
    nc, in_maps, col_of, N, MEM = _build(inputs)
    kernel._nc = nc
    kernel._in_maps = in_maps
    res = run_bass_kernel_spmd(nc, in_maps, list(range(NCORES)))
    hT = np.concatenate(
        [res.results[c]["hout"].astype(np.float32) for c in range(NCORES)], 0)
    out = np.empty((N, MEM), np.float32)
    for node in range(N):
        out[node] = hT[:, col_of[node]]
    return out


# revision 56
# speedup vs baseline: 1.0363x; 1.0363x over previous
"""ChildSumTreeLSTM with relation transforms on 8 Trainium2 NeuronCores.

Layout: everything transposed (features on SBUF partitions, tree nodes on the
free dim), node columns in topological-wave order (= heap order for the
reference tree). Per wave of parents:
  hsum (DVE strided reduce over child cols) -> per-slot 0/1 column masks
  (input data) zero the wrong-rel columns of hsum -> rel-sharded PE passes of
  SBUF-cached fp8 (x16-scaled) weight blocks, all slots accumulating into one
  PSUM group -> contrib copied out with a 1/16 activation scale -> bf16
  AllReduce (disjoint supports, exact) gives every core the full ch_sum ->
  column-sharded iou/f gates (each core owns one 128-feature slice); the fh
  matmuls run on PE while the AllReduce is in flight -> small AllGather of the
  new h columns.  xi/xf are folded into the PSUM accumulations with an
  identity-weight matmul so the gate nonlinearities read PSUM directly (fewer
  cross-engine hops).  Waves whose full rel set is cheap to compute everywhere
  are replicated on all cores and skip the AllReduce.
All per-core differences are input data (weight shards, masks, bias slices),
so one Bass program runs SPMD on all 8 cores.  All rel weights are preloaded
into SBUF at kernel start; host-side pre-transposed layouts keep every big
DMA one descriptor per partition.
"""

import sys

sys.path.insert(0, "/opt/trn_rl_repo")

import numpy as np
import ml_dtypes

import concourse.bass as bass
import concourse.mybir as mybir
import concourse.tile as tile
from concourse.bass_utils import run_bass_kernel_spmd
from concourse.vector_clock import ScopedClock, VectorClock

BF16 = mybir.dt.bfloat16
FP8 = mybir.dt.float8e4
F32 = mybir.dt.float32
NP_BF16 = ml_dtypes.bfloat16
NP_FP8 = ml_dtypes.float8_e4m3
NCORES = 8
P = 128
WSCALE = 16.0  # rel weights stored as fp8(W * WSCALE); contrib scaled back

# This walrus build rejects >1 sem wait per instruction at the Tile exit
# drain; split the aggregated drain into one drain per proc.
def _split_drain_and_barrier(self, tick_clock, wait_clock):
    gc = tick_clock.global_clock
    n = len(gc)
    nonzero = [i for i in range(n) if gc[i] > 0]
    for j in nonzero:
        vec = VectorClock([gc[i] if i == j else 0 for i in range(n)])
        d = self.nc.sync.drain()
        wait_clock.add_sem_waits(d.ins, ScopedClock({None: vec}))
    if not nonzero:
        d = self.nc.sync.drain()
        wait_clock.add_sem_waits(d.ins, ScopedClock({None: gc.copy()}))
    self.nc.all_engine_barrier()
    assert self.sems is not None
    popped = self.nc._tile_sem_poison_stack.pop()
    assert popped is self._sem_poison
    self.nc.clear_and_free_semaphores(list(self.sems.allocated().values()))
    self.nc.all_engine_barrier()


tile.TileContext._drain_and_barrier = _split_drain_and_barrier


def _split_multi_waits(nc, limit=1):
    """Walrus here allows only one sem wait per instruction; hoist extras
    onto same-engine NOPs inserted right before the instruction."""
    for bb in nc.main_func.blocks:
        new_list = []
        for ins in bb.instructions:
            si = getattr(ins, "sync_info", None)
            if si is not None and si.on_wait and len(si.on_wait) > limit:
                waits = list(si.on_wait)
                for w in waits[:-limit]:
                    nop = mybir.InstNoOp(
                        name=nc.get_next_instruction_name(),
                        sync_info=mybir.SyncInfo(on_wait=[w], on_update=[]),
                        bass_nofuse=True,
                        engine=ins.engine,
                    )
                    nc.register_instruction(nop, overwrite=True)
                    new_list.append(nop)
                si.on_wait = waits[-limit:]
            new_list.append(ins)
        bb.instructions[:] = new_list


def _bf16(a):
    return np.ascontiguousarray(a.astype(NP_BF16))


def _wslot(mat):
    """[M, K] f32 -> [P, MC*KC, P] fp8 packed lhsT blocks:
    [p, m*KC+k, :] = (mat[mb, kb].T)[p] * WSCALE."""
    M, K = mat.shape
    MC, KC = M // P, K // P
    out = np.empty((P, MC * KC, P), NP_FP8)
    q = (mat.astype(np.float32) * WSCALE).astype(NP_FP8)
    for m in range(MC):
        for k in range(KC):
            out[:, m * KC + k, :] = q[m * P:(m + 1) * P, k * P:(k + 1) * P].T
    return out


def _plan(child_idx, rel_ids, Wrel):
    """Host-side planning: waves, column order, rel->core assignment, slots."""
    N, K = child_idx.shape
    eff_children = []
    wave = np.zeros(N, np.int32)
    for i in range(N):
        cs = [int(c) for c in child_idx[i] if 0 <= c < i]
        eff_children.append(cs)
        wave[i] = 1 + max((wave[c] for c in cs), default=-1)
    nwaves = int(wave.max()) + 1
    # column order: by (wave, descending node) -> for the reference heap tree
    # this is exactly heap order (col j = node N-1-j) keeping children of
    # consecutive parents contiguous.
    order = sorted(range(N), key=lambda i: (wave[i], -i))
    col_of = np.empty(N, np.int64)
    for j, node in enumerate(order):
        col_of[node] = j
    waves = []  # list of (p0, p1) col ranges
    j = 0
    for w in range(nwaves):
        cnt = int((wave == w).sum())
        waves.append((j, j + cnt))
        j += cnt

    ident = set()
    eye = np.eye(Wrel.shape[1], dtype=Wrel.dtype)
    for r in set(int(rel_ids[i]) for i in range(N)):
        if np.array_equal(Wrel[r], eye):
            ident.add(r)

    # per wave (>=1): rels present; identity rels are skipped only when the
    # whole wave is identity (then ch_sum == hsum, no matmul or exchange)
    wave_rels = []
    for w in range(1, nwaves):
        p0, p1 = waves[w]
        rels_all = set(int(rel_ids[order[j]]) for j in range(p0, p1))
        if rels_all <= ident:
            wave_rels.append([])
        else:
            wave_rels.append(sorted(rels_all))

    # static rel->core assignment, greedy balance on total appearances
    from collections import defaultdict
    count = defaultdict(int)
    for rels in wave_rels:
        for r in rels:
            count[r] += 1
    nw = len(wave_rels)
    loadw = [[0] * nw for _ in range(NCORES)]
    assign = {}
    for r in sorted(count, key=lambda r: -count[r]):
        pres = [wi for wi in range(nw) if r in wave_rels[wi]]
        best, bkey = 0, None
        for c in range(NCORES):
            key = (sum(loadw[c][wi] for wi in pres), sum(loadw[c]))
            if bkey is None or key < bkey:
                best, bkey = c, key
        assign[r] = best
        for wi in pres:
            loadw[best][wi] += 1

    # per wave: per-core slot lists, padded to n_s.  Waves whose extra
    # replication cost (every core computing every rel) is below the
    # AllReduce floor are replicated: each core then holds the full ch_sum
    # locally and the wave needs no collective exchange.
    wave_slots = []  # per internal wave: (n_s, slots[c], replicated)
    for rels in wave_rels:
        per_core = [[r for r in rels if assign[r] == c] for c in range(NCORES)]
        n_s = max((len(x) for x in per_core), default=0)
        replicated = 0 < len(rels) and (len(rels) - n_s) <= 3
        if replicated:
            per_core = [list(rels) for _ in range(NCORES)]
            n_s = len(rels)
        wave_slots.append((n_s, per_core, replicated))
    return dict(order=order, col_of=col_of, waves=waves, wave=wave,
                eff_children=eff_children, ident=ident,
                wave_slots=wave_slots, nwaves=nwaves)


def _build(inputs):
    x = np.asarray(inputs["x"], np.float32)
    Wrel = np.asarray(inputs["Wrel"], np.float32)
    ioux_w = np.asarray(inputs["ioux_w"], np.float32)
    ioux_b = np.asarray(inputs["ioux_b"], np.float32)
    iouh_w = np.asarray(inputs["iouh_w"], np.float32)
    iouh_b = np.asarray(inputs["iouh_b"], np.float32)
    fx_w = np.asarray(inputs["fx_w"], np.float32)
    fx_b = np.asarray(inputs["fx_b"], np.float32)
    fh_w = np.asarray(inputs["fh_w"], np.float32)
    fh_b = np.asarray(inputs["fh_b"], np.float32)
    child_idx = np.asarray(inputs["child_idx"], np.int32)
    rel_ids = np.asarray(inputs["rel_ids"], np.int32)

    N, IN_DIM = x.shape
    MEM = fh_w.shape[0]
    KC = MEM // P           # 8 feature chunks
    KX = IN_DIM // P        # 8 input chunks
    K = child_idx.shape[1]  # max children (4)
    NPAD = N + K + 4

    plan = _plan(child_idx, rel_ids, Wrel)
    order, col_of, waves = plan["order"], plan["col_of"], plan["waves"]
    eff_children, ident = plan["eff_children"], plan["ident"]
    wave_slots, nwaves = plan["wave_slots"], plan["nwaves"]

    # Child gather plan: for each internal wave, the flattened (parent-major)
    # child column sequence, decomposed into maximal +1-contiguous runs.
    # Missing children point at the zero pad columns starting at ZCOL.
    ZCOL = N
    child_col = np.full((N, K), -1, np.int64)
    for i in range(N):
        for kk, c in enumerate(eff_children[i]):
            child_col[i, kk] = col_of[c]
    wave_runs = []  # per internal wave: list of (dst_off, src_col, length)
    for w in range(1, nwaves):
        p0, p1 = waves[w]
        seq = []
        for j in range(p0, p1):
            for kk in range(K):
                c = child_col[order[j], kk]
                seq.append(int(c) if c >= 0 else ZCOL + kk)
        runs = []
        i0 = 0
        while i0 < len(seq):
            i1 = i0 + 1
            while i1 < len(seq) and seq[i1] == seq[i1 - 1] + 1:
                i1 += 1
            runs.append((i0, int(seq[i0]), i1 - i0))
            i0 = i1
        wave_runs.append(runs)

    # internal-wave column offsets for the xf replication buffer
    itot = 0
    woff = []  # per internal wave: offset into xfb_all (units of K cols)
    for w in range(1, nwaves):
        woff.append(itot)
        itot += waves[w][1] - waves[w][0]

    # ---- per-core host data -------------------------------------------------
    # all big tensors pre-arranged partition-first so each DMA is one
    # contiguous run per partition
    xT = np.ascontiguousarray(x[order].T)  # [IN_DIM, N] in column order
    xt_h = np.zeros((P, KX, N), NP_BF16)
    for k in range(KX):
        xt_h[:, k, :] = _bf16(xT[k * P:(k + 1) * P])

    S_total = sum(ns for ns, _, _ in wave_slots)
    MC = MEM // P
    S_alloc = max(S_total, 1)
    # partition-major flat layout: [P, S*64*128] so each slice DMA is one
    # contiguous run per partition (cheap descriptor generation)
    wslots = [np.zeros((P, S_alloc, MC * KC, P), NP_FP8) for _ in range(NCORES)]
    NMAX = max((waves[w][1] - waves[w][0]) for w in range(1, nwaves)) if nwaves > 1 else 1
    NBIG = max(p1 - p0 for p0, p1 in waves)
    PSN = 128  # psum column pad so each m-chunk slice stays inside one bank
    assert NMAX <= PSN and K * NMAX <= 512
    masks = [np.zeros((S_alloc, KC, NMAX), NP_BF16) for _ in range(NCORES)]
    soff = 0
    for wi, (ns, per_core, _rep) in enumerate(wave_slots):
        w = wi + 1
        p0, p1 = waves[w]
        n = p1 - p0
        for c in range(NCORES):
            for s, r in enumerate(per_core[c]):
                wslots[c][:, soff + s] = _wslot(Wrel[r])
                for t in range(n):
                    if int(rel_ids[order[p0 + t]]) == r:
                        masks[c][soff + s, :, t] = 1.0
        soff += ns

    iouxstat = [np.zeros((P, KX * 3, P), NP_BF16) for _ in range(NCORES)]
    iouhstat = [np.zeros((P, KC * 3, P), NP_BF16) for _ in range(NCORES)]
    fxstat = [np.zeros((P, KX, P), NP_BF16) for _ in range(NCORES)]
    fhstat = [np.zeros((P, KC, P), NP_BF16) for _ in range(NCORES)]
    b_xi = [np.zeros((3, P), np.float32) for _ in range(NCORES)]
    b_iou = [np.zeros((3, P), np.float32) for _ in range(NCORES)]
    b_xf = [np.zeros((P,), np.float32) for _ in range(NCORES)]
    b_fh = [np.zeros((P,), np.float32) for _ in range(NCORES)]
    for c in range(NCORES):
        rows = slice(c * P, (c + 1) * P)
        for g in range(3):
            gr = slice(g * MEM + c * P, g * MEM + (c + 1) * P)
            b_xi[c][g] = ioux_b[gr]
            b_iou[c][g] = iouh_b[gr]
            for k in range(KX):
                iouxstat[c][:, k * 3 + g, :] = _bf16(
                    ioux_w[gr, k * P:(k + 1) * P].T)
            for k in range(KC):
                iouhstat[c][:, k * 3 + g, :] = _bf16(
                    iouh_w[gr, k * P:(k + 1) * P].T)
        b_xf[c] = fx_b[rows]
        b_fh[c] = fh_b[rows]
        for k in range(KX):
            fxstat[c][:, k, :] = _bf16(fx_w[rows, k * P:(k + 1) * P].T)
        for k in range(KC):
            fhstat[c][:, k, :] = _bf16(fh_w[rows, k * P:(k + 1) * P].T)
    eye_h = _bf16(np.eye(P, dtype=np.float32))

    # ---- build program ------------------------------------------------------
    nc = bass.Bass("TRN2", target_bir_lowering=False, debug=False,
                   num_devices=NCORES)
    d_ws = nc.dram_tensor("wslots", [P, S_alloc, MC * KC, P], FP8,
                          kind="ExternalInput")
    masks_x = [np.ascontiguousarray(
        np.broadcast_to(m[None], (P,) + m.shape)) for m in masks]
    d_mask = nc.dram_tensor("masks", list(masks_x[0].shape), BF16,
                            kind="ExternalInput")
    d_xt = nc.dram_tensor("xt", [P, KX, N], BF16, kind="ExternalInput")
    d_iouxs = nc.dram_tensor("iouxstat", [P, KX * 3, P], BF16, kind="ExternalInput")
    d_iouhs = nc.dram_tensor("iouhstat", [P, KC * 3, P], BF16, kind="ExternalInput")
    d_fxs = nc.dram_tensor("fxstat", [P, KX, P], BF16, kind="ExternalInput")
    d_fhs = nc.dram_tensor("fhstat", [P, KC, P], BF16, kind="ExternalInput")
    d_eye = nc.dram_tensor("eye", [P, P], BF16, kind="ExternalInput")
    d_bxi = nc.dram_tensor("b_xi", [3, P], F32, kind="ExternalInput")
    d_biou = nc.dram_tensor("b_iou", [3, P], F32, kind="ExternalInput")
    d_bxf = nc.dram_tensor("b_xf", [P], F32, kind="ExternalInput")
    d_bfh = nc.dram_tensor("b_fh", [P], F32, kind="ExternalInput")
    d_hout = nc.dram_tensor("hout", [P, N], BF16, kind="ExternalOutput")

    with tile.TileContext(nc, num_cores=NCORES) as tc:
        with (
            tc.tile_pool(name="const", bufs=1) as cpool,
            tc.tile_pool(name="state", bufs=1) as spool,
            tc.tile_pool(name="work", bufs=1) as wk,
            tc.tile_pool(name="mselp", bufs=1) as mselp,
            tc.tile_pool(name="psmm", bufs=1, space="PSUM") as pp,
            tc.tile_pool(name="psg", bufs=2, space="PSUM") as pg,
            tc.tile_pool(name="dram", bufs=2, space="DRAM") as dp,
        ):
            # constants needed for the leaf phase first
            xt = cpool.tile([P, KX, N], BF16)
            nc.sync.dma_start(xt[:], d_xt.ap())
            iouxs = cpool.tile([P, KX * 3, P], BF16)
            nc.sync.dma_start(iouxs[:], d_iouxs.ap())
            fxs = cpool.tile([P, KX, P], BF16)
            nc.sync.dma_start(fxs[:], d_fxs.ap())
            eye = cpool.tile([P, P], BF16)
            nc.sync.dma_start(eye[:], d_eye.ap())
            bxi = cpool.tile([P, 3], F32)
            nc.sync.dma_start(bxi[:], d_bxi.ap().rearrange("g p -> p g"))
            biou = cpool.tile([P, 3], F32)
            nc.sync.dma_start(biou[:], d_biou.ap().rearrange("g p -> p g"))
            bxf = cpool.tile([P, 1], F32)
            nc.sync.dma_start(bxf[:], d_bxf.ap().rearrange("(p one) -> p one", one=1))
            bfh = cpool.tile([P, 1], F32)
            nc.sync.dma_start(bfh[:], d_bfh.ap().rearrange("(p one) -> p one", one=1))
            # combined xi+iou bias for the leaf gates (they read the raw
            # ioux-matmul psum directly)
            bxiou = cpool.tile([P, 3], F32)
            nc.vector.tensor_add(bxiou[:], bxi[:], biou[:])

            # bulk prefetch tiles (DMAs issued later, on the Activation
            # HW-DGE ring, so the SP ring stays clean for latency-critical
            # staging around the collectives)
            iouhs = cpool.tile([P, KC * 3, P], BF16)
            fhs = cpool.tile([P, KC, P], BF16)
            msk = cpool.tile([P, S_alloc, KC, NMAX], BF16)
            wrel = cpool.tile([P, S_alloc * MC * KC, P], FP8)

            # state
            h_bf = spool.tile([P, KC, NPAD], BF16)
            nc.vector.memset(h_bf[:], 0.0)
            c_sl = spool.tile([P, NPAD], F32)
            nc.vector.memset(c_sl[:], 0.0)
            h_sl = spool.tile([P, N], BF16)
            xi_bf = spool.tile([P, 3, N], BF16)
            xf_bf = spool.tile([P, N], BF16)
            xfb_all = spool.tile([P, K * max(itot, 1)], BF16)

            ACT = mybir.ActivationFunctionType

            def gates(p0, n, psi, psf=None, cc_main=None, cc_tail=None,
                      main_ln=0, nch=0, bias_t=None):
                """Column-sharded gate math for parents at cols [p0, p0+n).
                psi: [P,3,n] PSUM accumulation including xi (via eye matmul,
                or raw ioux psum for leaves with bias_t=bxiou).
                psf: [P,nch] PSUM fh+xf accumulation, or None for leaves.
                cc_main: direct c_sl AP for the contiguous child block,
                cc_tail: staged c for the remaining child cols."""
                bt = biou if bias_t is None else bias_t
                # the whole f path only needs psf (ready before the
                # AllReduce) — emit it first so it runs during the AR wait
                # instead of queueing behind the psi-dependent activations
                if psf is not None:
                    fsb = wk.tile([P, K * NMAX], F32, tag="fsb")
                    nc.scalar.activation(fsb[:, :nch], psf, ACT.Sigmoid,
                                         bias=bfh[:, 0:1])
                    if main_ln:
                        nc.vector.tensor_mul(fsb[:, :main_ln],
                                             fsb[:, :main_ln], cc_main)
                    if nch > main_ln:
                        nc.vector.tensor_mul(fsb[:, main_ln:nch],
                                             fsb[:, main_ln:nch], cc_tail)
                    fc = wk.tile([P, NMAX], F32, tag="fc")
                    nc.vector.tensor_reduce(
                        fc[:, :n],
                        fsb[:, :nch].rearrange("p (n k) -> p n k", k=K),
                        axis=mybir.AxisListType.X, op=mybir.AluOpType.add)
                ig = wk.tile([P, NBIG], F32, tag="ig")
                og = wk.tile([P, NBIG], F32, tag="og")
                ug = wk.tile([P, NBIG], F32, tag="ug")
                nc.scalar.activation(ig[:, :n], psi[:, 0, :n], ACT.Sigmoid,
                                     bias=bt[:, 0:1])
                nc.scalar.activation(og[:, :n], psi[:, 1, :n], ACT.Sigmoid,
                                     bias=bt[:, 1:2])
                nc.scalar.activation(ug[:, :n], psi[:, 2, :n], ACT.Tanh,
                                     bias=bt[:, 2:3])
                cn = wk.tile([P, NBIG], F32, tag="cn")
                nc.vector.tensor_mul(cn[:, :n], ig[:, :n], ug[:, :n])
                if psf is not None:
                    nc.vector.tensor_add(c_sl[:, p0:p0 + n], cn[:, :n],
                                         fc[:, :n])
                else:
                    nc.vector.tensor_copy(c_sl[:, p0:p0 + n], cn[:, :n])
                tct = wk.tile([P, NBIG], F32, tag="tct")
                nc.scalar.activation(tct[:, :n], c_sl[:, p0:p0 + n], ACT.Tanh)
                with nc.allow_low_precision(reason="h is published in bf16"):
                    nc.vector.tensor_mul(h_sl[:, p0:p0 + n], og[:, :n],
                                         tct[:, :n])

            def publish_h(p0, n):
                # staging DMAs ride the Pool queue (SWDGE) right before the
                # trigger: in-order issue and a much cheaper completion sem
                sfx = str(n)
                gin = dp.tile([P, n], BF16, tag="gin" + sfx)
                nc.gpsimd.dma_start(gin[:], h_sl[:, p0:p0 + n])
                gout = dp.tile([NCORES, P, n], BF16, tag="gout" + sfx,
                               addr_space="Shared")
                nc.gpsimd.collective_compute(
                    "AllGather", mybir.AluOpType.bypass,
                    ins=[gin.opt()], outs=[gout.opt()],
                    replica_groups=[list(range(NCORES))])
                nc.gpsimd.dma_start(
                    h_bf[:, :, p0:p0 + n],
                    gout[:, :, :n].rearrange("k p n -> p k n"))

            # ---- wave 0 fused with the xi/xf precompute: leaf chunks gate
            # straight off the ioux psum; internal chunks store xi/xf
            p0, p1 = waves[0]
            n0 = p1 - p0
            CCH = PSN
            for cc in range(0, N, CCH):
                ncc = min(CCH, N - cc)
                ps = pg.tile([P, 3, PSN], F32, tag="ps3")
                for g in range(3):
                    for k in range(KX):
                        nc.tensor.matmul(
                            ps[:, g, :ncc],
                            iouxs[:, k * 3 + g, :],
                            xt[:, k, cc:cc + ncc],
                            start=(k == 0), stop=(k == KX - 1))
                nl = max(0, min(ncc, n0 - cc))        # leading leaf cols
                if nl:
                    gates(cc, nl, ps, bias_t=bxiou)
                    nc.sync.dma_start(d_hout.ap()[:, cc:cc + nl],
                                      h_sl[:, cc:cc + nl])
                if nl < ncc:                          # internal cols
                    o = nl
                    for g in range(3):
                        nc.scalar.activation(
                            xi_bf[:, g, cc + o:cc + ncc], ps[:, g, o:ncc],
                            ACT.Identity, bias=bxi[:, g:g + 1])
                    psf0 = pg.tile([P, K * NMAX], F32, tag="psf")
                    for k in range(KX):
                        nc.tensor.matmul(
                            psf0[:, o:ncc], fxs[:, k, :],
                            xt[:, k, cc + o:cc + ncc],
                            start=(k == 0), stop=(k == KX - 1))
                    nc.scalar.activation(
                        xf_bf[:, cc + o:cc + ncc], psf0[:, o:ncc],
                        ACT.Identity, bias=bxf[:, 0:1])
            publish_h(p0, n0)

            # xf replicated 4x per child slot for every internal wave
            # (feeds the f-gate psum via an identity matmul); off critical path
            for w in range(1, nwaves):
                pw0, pw1 = waves[w]
                nw = pw1 - pw0
                off = woff[w - 1] * K
                v = xfb_all[:, off:off + nw * K].rearrange(
                    "p (n k) -> p n k", k=K)
                for kk in range(K):
                    nc.vector.tensor_copy(
                        v[:, :, kk:kk + 1],
                        xf_bf[:, pw0:pw0 + nw].rearrange(
                            "p (n one) -> p n one", one=1))

            # bulk prefetch on the Activation HW-DGE ring, emitted after the
            # leaf-phase Act work so it doesn't delay the leaf gates; masks
            # are split per wave so wave 1's slice lands first
            soff_d = 0
            for wi in range(1, nwaves):
                ns_w = wave_slots[wi - 1][0]
                if ns_w:
                    nc.scalar.dma_start(
                        msk[:, soff_d:soff_d + ns_w], d_mask.ap()[:, soff_d:soff_d + ns_w])
                soff_d += ns_w
            nc.scalar.dma_start(iouhs[:], d_iouhs.ap())
            nc.scalar.dma_start(fhs[:], d_fhs.ap())
            soff_d = 0
            for wi in range(1, nwaves):
                ns_w = wave_slots[wi - 1][0]
                if ns_w:
                    nc.scalar.dma_start(
                        wrel[:, soff_d * MC * KC:(soff_d + ns_w) * MC * KC, :],
                        d_ws.ap()[:, soff_d:soff_d + ns_w])
                soff_d += ns_w

            # ---- internal waves -------------------------------------------
            soff = 0
            for wi in range(1, nwaves):
                ns, per_core, rep = wave_slots[wi - 1]
                p0, p1 = waves[wi]
                n = p1 - p0
                nch = n * K
                # the heap column order makes the children of all but the
                # last parent one contiguous block in h_bf/c_sl: read it
                # directly; only the tail parent's cols go through staging
                runs = wave_runs[wi - 1]
                if runs and runs[0][0] == 0:
                    src0 = runs[0][1]
                    main_np = runs[0][2] // K
                else:
                    src0, main_np = 0, 0
                main_ln = main_np * K
                tail_ln = nch - main_ln
                hct = wk.tile([P, KC, K * NMAX], BF16, tag="hct")
                cct = wk.tile([P, K * NMAX], F32, tag="cct")
                for (dst, src, ln) in runs:
                    lo = max(dst, main_ln)
                    hi = dst + ln
                    if hi <= lo:
                        continue
                    o = lo - dst
                    nc.vector.tensor_copy(
                        cct[:, lo - main_ln:hi - main_ln],
                        c_sl[:, src + o:src + o + (hi - lo)])
                    nc.gpsimd.tensor_copy(
                        hct[:, :, lo - main_ln:hi - main_ln],
                        h_bf[:, :, src + o:src + o + (hi - lo)])
                # hsum over child cols (bf16 in, bf16 out); one 4D reduce
                # per source covers all feature chunks
                hsum_b = wk.tile([P, KC, NMAX], BF16, tag="hsumb")
                with nc.allow_low_precision(reason="4-term bf16 child sum"):
                    if main_np:
                        nc.vector.tensor_reduce(
                            hsum_b[:, :, :main_np],
                            h_bf[:, :, src0:src0 + main_ln].rearrange(
                                "p k (n c) -> p k n c", c=K),
                            axis=mybir.AxisListType.X,
                            op=mybir.AluOpType.add)
                    if tail_ln:
                        nc.vector.tensor_reduce(
                            hsum_b[:, :, main_np:n],
                            hct[:, :, :tail_ln].rearrange(
                                "p k (n c) -> p k n c", c=K),
                            axis=mybir.AxisListType.X,
                            op=mybir.AluOpType.add)

                all_id = (ns == 0)
                # replicated single-rel wave: every mask is all-ones, feed
                # hsum straight to the matmul (no msel op or edge)
                uniform = rep and ns == 1
                if not all_id:
                    psl = pp.tile([P, MC, PSN], F32, tag="psl")
                    msels = []
                    for s in range(ns):
                        if uniform:
                            msels.append(hsum_b)
                            continue
                        msel = mselp.tile([P, KC, NMAX], BF16,
                                          tag="msel" + str(s))
                        eng = nc.vector if s % 2 == 0 else nc.gpsimd
                        eng.tensor_mul(msel[:, :, :n], hsum_b[:, :, :n],
                                       msk[:, soff + s, :, :n])
                        msels.append(msel)
                    # m-outer so each psum region's accumulation group
                    # (spanning all slots) closes before the next opens
                    for m in range(MC):
                        for s in range(ns):
                            for k in range(KC):
                                nc.tensor.matmul(
                                    psl[:, m, :n],
                                    wrel[:, (soff + s) * MC * KC + m * KC + k, :],
                                    msels[s][:, k, :n],
                                    start=(s == 0 and k == 0),
                                    stop=(s == ns - 1 and k == KC - 1))
                    # scale fp8 weights back (1/WSCALE) on the psum read
                    cb = wk.tile([P, KC, n], BF16, tag="cb" + str(n))
                    nc.scalar.activation(
                        cb[:, :, :n], psl[:, :, :n], ACT.Identity,
                        scale=1.0 / WSCALE)
                    if rep:
                        rhs = cb     # every core computed the full ch_sum
                    else:
                        g1in = dp.tile([P, KC, n], BF16, tag="g1in" + str(n))
                        nc.gpsimd.dma_start(g1in[:], cb[:])
                        g1out = dp.tile([P, KC, n], BF16, tag="g1out" + str(n),
                                        addr_space="Shared")
                        # contributions have disjoint support (masked), so
                        # the bf16 CCE adds are exact
                        nc.gpsimd.collective_compute(
                            "AllReduce", mybir.AluOpType.add,
                            ins=[g1in.opt()], outs=[g1out.opt()],
                            replica_groups=[list(range(NCORES))])
                        chs_b = wk.tile([P, KC, n], BF16, tag="chsb" + str(n))
                        nc.gpsimd.dma_start(chs_b[:], g1out[:])
                        rhs = chs_b
                else:
                    rhs = hsum_b

                # fh matmuls first: they read h_bf/hct directly, so PE runs
                # them while the AllReduce is in flight; xf joins via eye mm
                psf = pg.tile([P, K * NMAX], F32, tag="psf")
                off = woff[wi - 1] * K
                if main_ln:
                    for k in range(KC):
                        nc.tensor.matmul(
                            psf[:, :main_ln], fhs[:, k, :],
                            h_bf[:, k, src0:src0 + main_ln],
                            start=(k == 0), stop=False)
                    nc.tensor.matmul(psf[:, :main_ln], eye[:],
                                     xfb_all[:, off:off + main_ln],
                                     start=False, stop=True)
                if tail_ln:
                    for k in range(KC):
                        nc.tensor.matmul(
                            psf[:, main_ln:nch], fhs[:, k, :],
                            hct[:, k, :tail_ln],
                            start=(k == 0), stop=False)
                    nc.tensor.matmul(psf[:, main_ln:nch], eye[:],
                                     xfb_all[:, off + main_ln:off + nch],
                                     start=False, stop=True)
                # iou matmuls (column-sharded); xi joins via eye matmul
                psi = pg.tile([P, 3, PSN], F32, tag="ps3")
                for g in range(3):
                    for k in range(KC):
                        nc.tensor.matmul(
                            psi[:, g, :n], iouhs[:, k * 3 + g, :],
                            rhs[:, k, :n],
                            start=(k == 0), stop=False)
                    nc.tensor.matmul(psi[:, g, :n], eye[:],
                                     xi_bf[:, g, p0:p0 + n],
                                     start=False, stop=True)
                gates(p0, n, psi, psf[:, :nch],
                      cc_main=(c_sl[:, src0:src0 + main_ln]
                               if main_ln else None),
                      cc_tail=(cct[:, :tail_ln] if tail_ln else None),
                      main_ln=main_ln, nch=nch)
                nc.sync.dma_start(d_hout.ap()[:, p0:p0 + n],
                                  h_sl[:, p0:p0 + n])
                if wi < nwaves - 1:
                    publish_h(p0, n)
                soff += ns

    in_maps = []
    for c in range(NCORES):
        in_maps.append({
            "wslots": wslots[c], "masks": masks_x[c],
            "xt": xt_h, "iouxstat": iouxstat[c], "iouhstat": iouhstat[c],
            "fxstat": fxstat[c], "fhstat": fhstat[c], "eye": eye_h,
            "b_xi": b_xi[c], "b_iou": b_iou[c], "b_xf": b_xf[c],
            "b_fh": b_fh[c],
        })
    _split_multi_waits(nc)
    return nc, in_maps, col_of, N, MEM


def kernel(**inputs):
    nc, in_maps, col_of, N, MEM = _build(inputs)
    kernel._nc = nc
    kernel._in_maps = in_maps
    res = run_bass_kernel_spmd(nc, in_maps, list(range(NCORES)))
    hT = np.concatenate(
        [res.results[c]["hout"].astype(np.float32) for c in range(NCORES)], 0)
    out = np.empty((N, MEM), np.float32)
    for node in range(N):
        out[node] = hT[:, col_of[node]]
    return out


# revision 59
# speedup vs baseline: 1.0794x; 1.0416x over previous
"""ChildSumTreeLSTM with relation transforms on 8 Trainium2 NeuronCores.

Layout: everything transposed (features on SBUF partitions, tree nodes on the
free dim), node columns in topological-wave order (= heap order for the
reference tree). Per wave of parents:
  hsum (DVE strided reduce over child cols) -> per-slot 0/1 column masks
  (input data) zero the wrong-rel columns of hsum -> rel-sharded PE passes of
  SBUF-cached fp8 (x16-scaled) weight blocks, all slots accumulating into one
  PSUM group -> contrib copied out with a 1/16 activation scale -> bf16
  AllReduce (disjoint supports, exact) gives every core the full ch_sum ->
  column-sharded iou/f gates (each core owns one 128-feature slice); the fh
  matmuls run on PE while the AllReduce is in flight -> small AllGather of the
  new h columns.  xi/xf are folded into the PSUM accumulations with an
  identity-weight matmul so the gate nonlinearities read PSUM directly (fewer
  cross-engine hops).  Waves whose full rel set is cheap to compute everywhere
  are replicated on all cores and skip the AllReduce.
All per-core differences are input data (weight shards, masks, bias slices),
so one Bass program runs SPMD on all 8 cores.  All rel weights are preloaded
into SBUF at kernel start; host-side pre-transposed layouts keep every big
DMA one descriptor per partition.
"""

import sys

sys.path.insert(0, "/opt/trn_rl_repo")

import numpy as np
import ml_dtypes

import concourse.bass as bass
import concourse.mybir as mybir
import concourse.tile as tile
from concourse.bass_utils import run_bass_kernel_spmd
from concourse.vector_clock import ScopedClock, VectorClock

BF16 = mybir.dt.bfloat16
FP8 = mybir.dt.float8e4
F32 = mybir.dt.float32
NP_BF16 = ml_dtypes.bfloat16
NP_FP8 = ml_dtypes.float8_e4m3
NCORES = 8
P = 128
WSCALE = 16.0  # rel weights stored as fp8(W * WSCALE); contrib scaled back

# This walrus build rejects >1 sem wait per instruction at the Tile exit
# drain; split the aggregated drain into one drain per proc.
def _split_drain_and_barrier(self, tick_clock, wait_clock):
    gc = tick_clock.global_clock
    n = len(gc)
    nonzero = [i for i in range(n) if gc[i] > 0]
    for j in nonzero:
        vec = VectorClock([gc[i] if i == j else 0 for i in range(n)])
        d = self.nc.sync.drain()
        wait_clock.add_sem_waits(d.ins, ScopedClock({None: vec}))
    if not nonzero:
        d = self.nc.sync.drain()
        wait_clock.add_sem_waits(d.ins, ScopedClock({None: gc.copy()}))
    self.nc.all_engine_barrier()
    assert self.sems is not None
    popped = self.nc._tile_sem_poison_stack.pop()
    assert popped is self._sem_poison
    self.nc.clear_and_free_semaphores(list(self.sems.allocated().values()))
    self.nc.all_engine_barrier()


tile.TileContext._drain_and_barrier = _split_drain_and_barrier


def _split_multi_waits(nc, limit=1):
    """Walrus here allows only one sem wait per instruction; hoist extras
    onto same-engine NOPs inserted right before the instruction."""
    for bb in nc.main_func.blocks:
        new_list = []
        for ins in bb.instructions:
            si = getattr(ins, "sync_info", None)
            if si is not None and si.on_wait and len(si.on_wait) > limit:
                waits = list(si.on_wait)
                for w in waits[:-limit]:
                    nop = mybir.InstNoOp(
                        name=nc.get_next_instruction_name(),
                        sync_info=mybir.SyncInfo(on_wait=[w], on_update=[]),
                        bass_nofuse=True,
                        engine=ins.engine,
                    )
                    nc.register_instruction(nop, overwrite=True)
                    new_list.append(nop)
                si.on_wait = waits[-limit:]
            new_list.append(ins)
        bb.instructions[:] = new_list


def _bf16(a):
    return np.ascontiguousarray(a.astype(NP_BF16))


def _wslot(mat):
    """[M, K] f32 -> [P, MC*KC, P] fp8 packed lhsT blocks:
    [p, m*KC+k, :] = (mat[mb, kb].T)[p] * WSCALE."""
    M, K = mat.shape
    MC, KC = M // P, K // P
    out = np.empty((P, MC * KC, P), NP_FP8)
    q = (mat.astype(np.float32) * WSCALE).astype(NP_FP8)
    for m in range(MC):
        for k in range(KC):
            out[:, m * KC + k, :] = q[m * P:(m + 1) * P, k * P:(k + 1) * P].T
    return out


def _plan(child_idx, rel_ids, Wrel):
    """Host-side planning: waves, column order, rel->core assignment, slots."""
    N, K = child_idx.shape
    eff_children = []
    wave = np.zeros(N, np.int32)
    for i in range(N):
        cs = [int(c) for c in child_idx[i] if 0 <= c < i]
        eff_children.append(cs)
        wave[i] = 1 + max((wave[c] for c in cs), default=-1)
    nwaves = int(wave.max()) + 1
    # column order: by (wave, descending node) -> for the reference heap tree
    # this is exactly heap order (col j = node N-1-j) keeping children of
    # consecutive parents contiguous.
    order = sorted(range(N), key=lambda i: (wave[i], -i))
    col_of = np.empty(N, np.int64)
    for j, node in enumerate(order):
        col_of[node] = j
    waves = []  # list of (p0, p1) col ranges
    j = 0
    for w in range(nwaves):
        cnt = int((wave == w).sum())
        waves.append((j, j + cnt))
        j += cnt

    ident = set()
    eye = np.eye(Wrel.shape[1], dtype=Wrel.dtype)
    for r in set(int(rel_ids[i]) for i in range(N)):
        if np.array_equal(Wrel[r], eye):
            ident.add(r)

    # per wave (>=1): rels present; identity rels are skipped only when the
    # whole wave is identity (then ch_sum == hsum, no matmul or exchange)
    wave_rels = []
    for w in range(1, nwaves):
        p0, p1 = waves[w]
        rels_all = set(int(rel_ids[order[j]]) for j in range(p0, p1))
        if rels_all <= ident:
            wave_rels.append([])
        else:
            wave_rels.append(sorted(rels_all))

    # static rel->core assignment, greedy balance on total appearances
    from collections import defaultdict
    count = defaultdict(int)
    for rels in wave_rels:
        for r in rels:
            count[r] += 1
    nw = len(wave_rels)
    loadw = [[0] * nw for _ in range(NCORES)]
    assign = {}
    for r in sorted(count, key=lambda r: -count[r]):
        pres = [wi for wi in range(nw) if r in wave_rels[wi]]
        best, bkey = 0, None
        for c in range(NCORES):
            key = (sum(loadw[c][wi] for wi in pres), sum(loadw[c]))
            if bkey is None or key < bkey:
                best, bkey = c, key
        assign[r] = best
        for wi in pres:
            loadw[best][wi] += 1

    # per wave: per-core slot lists, padded to n_s.  Waves whose extra
    # replication cost (every core computing every rel) is below the
    # AllReduce floor are replicated: each core then holds the full ch_sum
    # locally and the wave needs no collective exchange.
    wave_slots = []  # per internal wave: (n_s, slots[c], replicated)
    for rels in wave_rels:
        per_core = [[r for r in rels if assign[r] == c] for c in range(NCORES)]
        n_s = max((len(x) for x in per_core), default=0)
        replicated = 0 < len(rels) and (len(rels) - n_s) <= 3
        if replicated:
            per_core = [list(rels) for _ in range(NCORES)]
            n_s = len(rels)
        wave_slots.append((n_s, per_core, replicated))
    return dict(order=order, col_of=col_of, waves=waves, wave=wave,
                eff_children=eff_children, ident=ident,
                wave_slots=wave_slots, nwaves=nwaves)


def _build(inputs):
    x = np.asarray(inputs["x"], np.float32)
    Wrel = np.asarray(inputs["Wrel"], np.float32)
    ioux_w = np.asarray(inputs["ioux_w"], np.float32)
    ioux_b = np.asarray(inputs["ioux_b"], np.float32)
    iouh_w = np.asarray(inputs["iouh_w"], np.float32)
    iouh_b = np.asarray(inputs["iouh_b"], np.float32)
    fx_w = np.asarray(inputs["fx_w"], np.float32)
    fx_b = np.asarray(inputs["fx_b"], np.float32)
    fh_w = np.asarray(inputs["fh_w"], np.float32)
    fh_b = np.asarray(inputs["fh_b"], np.float32)
    child_idx = np.asarray(inputs["child_idx"], np.int32)
    rel_ids = np.asarray(inputs["rel_ids"], np.int32)

    N, IN_DIM = x.shape
    MEM = fh_w.shape[0]
    KC = MEM // P           # 8 feature chunks
    KX = IN_DIM // P        # 8 input chunks
    K = child_idx.shape[1]  # max children (4)
    NPAD = N + K + 4

    plan = _plan(child_idx, rel_ids, Wrel)
    order, col_of, waves = plan["order"], plan["col_of"], plan["waves"]
    eff_children, ident = plan["eff_children"], plan["ident"]
    wave_slots, nwaves = plan["wave_slots"], plan["nwaves"]

    # Child gather plan: for each internal wave, the flattened (parent-major)
    # child column sequence, decomposed into maximal +1-contiguous runs.
    # Missing children point at the zero pad columns starting at ZCOL.
    ZCOL = N
    child_col = np.full((N, K), -1, np.int64)
    for i in range(N):
        for kk, c in enumerate(eff_children[i]):
            child_col[i, kk] = col_of[c]
    wave_runs = []  # per internal wave: list of (dst_off, src_col, length)
    for w in range(1, nwaves):
        p0, p1 = waves[w]
        seq = []
        for j in range(p0, p1):
            for kk in range(K):
                c = child_col[order[j], kk]
                seq.append(int(c) if c >= 0 else ZCOL + kk)
        runs = []
        i0 = 0
        while i0 < len(seq):
            i1 = i0 + 1
            while i1 < len(seq) and seq[i1] == seq[i1 - 1] + 1:
                i1 += 1
            runs.append((i0, int(seq[i0]), i1 - i0))
            i0 = i1
        wave_runs.append(runs)

    # internal-wave column offsets for the xf replication buffer
    itot = 0
    woff = []  # per internal wave: offset into xfb_all (units of K cols)
    for w in range(1, nwaves):
        woff.append(itot)
        itot += waves[w][1] - waves[w][0]

    # ---- per-core host data -------------------------------------------------
    # all big tensors pre-arranged partition-first so each DMA is one
    # contiguous run per partition
    xT = np.ascontiguousarray(x[order].T)  # [IN_DIM, N] in column order
    xt_h = np.zeros((P, KX, N), NP_BF16)
    for k in range(KX):
        xt_h[:, k, :] = _bf16(xT[k * P:(k + 1) * P])

    S_total = sum(ns for ns, _, _ in wave_slots)
    MC = MEM // P
    S_alloc = max(S_total, 1)
    # partition-major flat layout: [P, S*64*128] so each slice DMA is one
    # contiguous run per partition (cheap descriptor generation)
    wslots = [np.zeros((P, S_alloc, MC * KC, P), NP_FP8) for _ in range(NCORES)]
    NMAX = max((waves[w][1] - waves[w][0]) for w in range(1, nwaves)) if nwaves > 1 else 1
    NBIG = max(p1 - p0 for p0, p1 in waves)
    PSN = 128  # psum column pad so each m-chunk slice stays inside one bank
    assert NMAX <= PSN and K * NMAX <= 512
    masks = [np.zeros((S_alloc, KC, NMAX), NP_BF16) for _ in range(NCORES)]
    soff = 0
    for wi, (ns, per_core, _rep) in enumerate(wave_slots):
        w = wi + 1
        p0, p1 = waves[w]
        n = p1 - p0
        for c in range(NCORES):
            for s, r in enumerate(per_core[c]):
                wslots[c][:, soff + s] = _wslot(Wrel[r])
                for t in range(n):
                    if int(rel_ids[order[p0 + t]]) == r:
                        masks[c][soff + s, :, t] = 1.0
        soff += ns

    iouxstat = [np.zeros((P, KX * 3, P), NP_BF16) for _ in range(NCORES)]
    iouhstat = [np.zeros((P, KC * 3, P), NP_BF16) for _ in range(NCORES)]
    fxstat = [np.zeros((P, KX, P), NP_BF16) for _ in range(NCORES)]
    fhstat = [np.zeros((P, KC, P), NP_BF16) for _ in range(NCORES)]
    b_xi = [np.zeros((3, P), np.float32) for _ in range(NCORES)]
    b_iou = [np.zeros((3, P), np.float32) for _ in range(NCORES)]
    b_xf = [np.zeros((P,), np.float32) for _ in range(NCORES)]
    b_fh = [np.zeros((P,), np.float32) for _ in range(NCORES)]
    for c in range(NCORES):
        rows = slice(c * P, (c + 1) * P)
        for g in range(3):
            gr = slice(g * MEM + c * P, g * MEM + (c + 1) * P)
            b_xi[c][g] = ioux_b[gr]
            b_iou[c][g] = iouh_b[gr]
            for k in range(KX):
                iouxstat[c][:, k * 3 + g, :] = _bf16(
                    ioux_w[gr, k * P:(k + 1) * P].T)
            for k in range(KC):
                iouhstat[c][:, k * 3 + g, :] = _bf16(
                    iouh_w[gr, k * P:(k + 1) * P].T)
        b_xf[c] = fx_b[rows]
        b_fh[c] = fh_b[rows]
        for k in range(KX):
            fxstat[c][:, k, :] = _bf16(fx_w[rows, k * P:(k + 1) * P].T)
        for k in range(KC):
            fhstat[c][:, k, :] = _bf16(fh_w[rows, k * P:(k + 1) * P].T)
    eye_h = _bf16(np.eye(P, dtype=np.float32))

    # ---- build program ------------------------------------------------------
    nc = bass.Bass("TRN2", target_bir_lowering=False, debug=False,
                   num_devices=NCORES)
    d_ws = nc.dram_tensor("wslots", [P, S_alloc, MC * KC, P], FP8,
                          kind="ExternalInput")
    masks_x = [np.ascontiguousarray(
        np.broadcast_to(m[None], (P,) + m.shape)) for m in masks]
    d_mask = nc.dram_tensor("masks", list(masks_x[0].shape), BF16,
                            kind="ExternalInput")
    d_xt = nc.dram_tensor("xt", [P, KX, N], BF16, kind="ExternalInput")
    d_iouxs = nc.dram_tensor("iouxstat", [P, KX * 3, P], BF16, kind="ExternalInput")
    d_iouhs = nc.dram_tensor("iouhstat", [P, KC * 3, P], BF16, kind="ExternalInput")
    d_fxs = nc.dram_tensor("fxstat", [P, KX, P], BF16, kind="ExternalInput")
    d_fhs = nc.dram_tensor("fhstat", [P, KC, P], BF16, kind="ExternalInput")
    d_eye = nc.dram_tensor("eye", [P, P], BF16, kind="ExternalInput")
    d_bxi = nc.dram_tensor("b_xi", [3, P], F32, kind="ExternalInput")
    d_biou = nc.dram_tensor("b_iou", [3, P], F32, kind="ExternalInput")
    d_bxf = nc.dram_tensor("b_xf", [P], F32, kind="ExternalInput")
    d_bfh = nc.dram_tensor("b_fh", [P], F32, kind="ExternalInput")
    d_hout = nc.dram_tensor("hout", [P, N], BF16, kind="ExternalOutput")

    with tile.TileContext(nc, num_cores=NCORES) as tc:
        with (
            tc.tile_pool(name="const", bufs=1) as cpool,
            tc.tile_pool(name="state", bufs=1) as spool,
            tc.tile_pool(name="work", bufs=1) as wk,
            tc.tile_pool(name="mselp", bufs=1) as mselp,
            tc.tile_pool(name="psmm", bufs=1, space="PSUM") as pp,
            tc.tile_pool(name="psg", bufs=2, space="PSUM") as pg,
            tc.tile_pool(name="dram", bufs=2, space="DRAM") as dp,
        ):
            # constants needed for the leaf phase first
            xt = cpool.tile([P, KX, N], BF16)
            nc.sync.dma_start(xt[:], d_xt.ap())
            iouxs = cpool.tile([P, KX * 3, P], BF16)
            nc.sync.dma_start(iouxs[:], d_iouxs.ap())
            fxs = cpool.tile([P, KX, P], BF16)
            nc.sync.dma_start(fxs[:], d_fxs.ap())
            eye = cpool.tile([P, P], BF16)
            nc.sync.dma_start(eye[:], d_eye.ap())
            bxi = cpool.tile([P, 3], F32)
            nc.sync.dma_start(bxi[:], d_bxi.ap().rearrange("g p -> p g"))
            biou = cpool.tile([P, 3], F32)
            nc.sync.dma_start(biou[:], d_biou.ap().rearrange("g p -> p g"))
            bxf = cpool.tile([P, 1], F32)
            nc.sync.dma_start(bxf[:], d_bxf.ap().rearrange("(p one) -> p one", one=1))
            bfh = cpool.tile([P, 1], F32)
            nc.sync.dma_start(bfh[:], d_bfh.ap().rearrange("(p one) -> p one", one=1))
            # combined xi+iou bias for the leaf gates (they read the raw
            # ioux-matmul psum directly)
            bxiou = cpool.tile([P, 3], F32)
            nc.vector.tensor_add(bxiou[:], bxi[:], biou[:])

            # bulk prefetch tiles (DMAs issued later, on the Activation
            # HW-DGE ring, so the SP ring stays clean for latency-critical
            # staging around the collectives)
            iouhs = cpool.tile([P, KC * 3, P], BF16)
            fhs = cpool.tile([P, KC, P], BF16)
            msk = cpool.tile([P, S_alloc, KC, NMAX], BF16)
            wrel = cpool.tile([P, S_alloc * MC * KC, P], FP8)

            # state
            h_bf = spool.tile([P, KC, NPAD], BF16)
            nc.vector.memset(h_bf[:], 0.0)
            c_sl = spool.tile([P, NPAD], F32)
            nc.vector.memset(c_sl[:], 0.0)
            h_sl = spool.tile([P, N], BF16)
            xi_bf = spool.tile([P, 3, N], BF16)
            xf_bf = spool.tile([P, N], BF16)
            xfb_all = spool.tile([P, K * max(itot, 1)], BF16)

            ACT = mybir.ActivationFunctionType

            def gates(p0, n, psi, psf=None, cc_main=None, cc_tail=None,
                      main_ln=0, nch=0, bias_t=None):
                """Column-sharded gate math for parents at cols [p0, p0+n).
                psi: [P,3,n] PSUM accumulation including xi (via eye matmul,
                or raw ioux psum for leaves with bias_t=bxiou).
                psf: [P,nch] PSUM fh+xf accumulation, or None for leaves.
                cc_main: direct c_sl AP for the contiguous child block,
                cc_tail: staged c for the remaining child cols."""
                bt = biou if bias_t is None else bias_t
                # the whole f path only needs psf (ready before the
                # AllReduce) — emit it first so it runs during the AR wait
                # instead of queueing behind the psi-dependent activations
                if psf is not None:
                    fsb = wk.tile([P, K * NMAX], F32, tag="fsb")
                    nc.scalar.activation(fsb[:, :nch], psf, ACT.Sigmoid,
                                         bias=bfh[:, 0:1])
                    if main_ln:
                        nc.vector.tensor_mul(fsb[:, :main_ln],
                                             fsb[:, :main_ln], cc_main)
                    if nch > main_ln:
                        nc.vector.tensor_mul(fsb[:, main_ln:nch],
                                             fsb[:, main_ln:nch], cc_tail)
                    fc = wk.tile([P, NMAX], F32, tag="fc")
                    nc.vector.tensor_reduce(
                        fc[:, :n],
                        fsb[:, :nch].rearrange("p (n k) -> p n k", k=K),
                        axis=mybir.AxisListType.X, op=mybir.AluOpType.add)
                igog = wk.tile([P, 2, NBIG], F32, tag="igog")
                ug = wk.tile([P, NBIG], F32, tag="ug")
                if bias_t is None:
                    # internal wave: bias already folded into xi_bf, so
                    # i and o share one fused sigmoid op
                    nc.scalar.activation(igog[:, :, :n], psi[:, 0:2, :n],
                                         ACT.Sigmoid)
                    nc.scalar.activation(ug[:, :n], psi[:, 2, :n], ACT.Tanh)
                else:
                    nc.scalar.activation(igog[:, 0, :n], psi[:, 0, :n],
                                         ACT.Sigmoid, bias=bt[:, 0:1])
                    nc.scalar.activation(igog[:, 1, :n], psi[:, 1, :n],
                                         ACT.Sigmoid, bias=bt[:, 1:2])
                    nc.scalar.activation(ug[:, :n], psi[:, 2, :n], ACT.Tanh,
                                         bias=bt[:, 2:3])
                ig = igog[:, 0, :]
                og = igog[:, 1, :]
                cn = wk.tile([P, NBIG], F32, tag="cn")
                nc.vector.tensor_mul(cn[:, :n], ig[:, :n], ug[:, :n])
                if psf is not None:
                    nc.vector.tensor_add(c_sl[:, p0:p0 + n], cn[:, :n],
                                         fc[:, :n])
                else:
                    nc.vector.tensor_copy(c_sl[:, p0:p0 + n], cn[:, :n])
                tct = wk.tile([P, NBIG], F32, tag="tct")
                nc.scalar.activation(tct[:, :n], c_sl[:, p0:p0 + n], ACT.Tanh)
                with nc.allow_low_precision(reason="h is published in bf16"):
                    nc.vector.tensor_mul(h_sl[:, p0:p0 + n], og[:, :n],
                                         tct[:, :n])

            def publish_h(p0, n):
                # staging DMAs ride the Pool queue (SWDGE) right before the
                # trigger: in-order issue and a much cheaper completion sem
                sfx = str(n)
                gin = dp.tile([P, n], BF16, tag="gin" + sfx)
                nc.gpsimd.dma_start(gin[:], h_sl[:, p0:p0 + n])
                gout = dp.tile([NCORES, P, n], BF16, tag="gout" + sfx,
                               addr_space="Shared")
                nc.gpsimd.collective_compute(
                    "AllGather", mybir.AluOpType.bypass,
                    ins=[gin.opt()], outs=[gout.opt()],
                    replica_groups=[list(range(NCORES))])
                nc.gpsimd.dma_start(
                    h_bf[:, :, p0:p0 + n],
                    gout[:, :, :n].rearrange("k p n -> p k n"))

            # ---- wave 0 fused with the xi/xf precompute: leaf chunks gate
            # straight off the ioux psum; internal chunks store xi/xf
            p0, p1 = waves[0]
            n0 = p1 - p0
            CCH = PSN
            for cc in range(0, N, CCH):
                ncc = min(CCH, N - cc)
                ps = pg.tile([P, 3, PSN], F32, tag="ps3")
                for g in range(3):
                    for k in range(KX):
                        nc.tensor.matmul(
                            ps[:, g, :ncc],
                            iouxs[:, k * 3 + g, :],
                            xt[:, k, cc:cc + ncc],
                            start=(k == 0), stop=(k == KX - 1))
                nl = max(0, min(ncc, n0 - cc))        # leading leaf cols
                if nl:
                    gates(cc, nl, ps, bias_t=bxiou)
                    nc.sync.dma_start(d_hout.ap()[:, cc:cc + nl],
                                      h_sl[:, cc:cc + nl])
                if nl < ncc:                          # internal cols
                    o = nl
                    for g in range(3):
                        # fold the iou bias in too: the wave gate
                        # activations then need no bias at all
                        nc.scalar.activation(
                            xi_bf[:, g, cc + o:cc + ncc], ps[:, g, o:ncc],
                            ACT.Identity, bias=bxiou[:, g:g + 1])
                    psf0 = pg.tile([P, K * NMAX], F32, tag="psf")
                    for k in range(KX):
                        nc.tensor.matmul(
                            psf0[:, o:ncc], fxs[:, k, :],
                            xt[:, k, cc + o:cc + ncc],
                            start=(k == 0), stop=(k == KX - 1))
                    nc.scalar.activation(
                        xf_bf[:, cc + o:cc + ncc], psf0[:, o:ncc],
                        ACT.Identity, bias=bxf[:, 0:1])
            publish_h(p0, n0)

            # xf replicated 4x per child slot for every internal wave
            # (feeds the f-gate psum via an identity matmul); off critical path
            for w in range(1, nwaves):
                pw0, pw1 = waves[w]
                nw = pw1 - pw0
                off = woff[w - 1] * K
                v = xfb_all[:, off:off + nw * K].rearrange(
                    "p (n k) -> p n k", k=K)
                for kk in range(K):
                    nc.vector.tensor_copy(
                        v[:, :, kk:kk + 1],
                        xf_bf[:, pw0:pw0 + nw].rearrange(
                            "p (n one) -> p n one", one=1))

            # bulk prefetch on the Activation HW-DGE ring, emitted after the
            # leaf-phase Act work so it doesn't delay the leaf gates; masks
            # are split per wave so wave 1's slice lands first
            soff_d = 0
            for wi in range(1, nwaves):
                ns_w = wave_slots[wi - 1][0]
                if ns_w:
                    nc.scalar.dma_start(
                        msk[:, soff_d:soff_d + ns_w], d_mask.ap()[:, soff_d:soff_d + ns_w])
                soff_d += ns_w
            nc.scalar.dma_start(iouhs[:], d_iouhs.ap())
            nc.scalar.dma_start(fhs[:], d_fhs.ap())
            soff_d = 0
            for wi in range(1, nwaves):
                ns_w = wave_slots[wi - 1][0]
                if ns_w:
                    nc.scalar.dma_start(
                        wrel[:, soff_d * MC * KC:(soff_d + ns_w) * MC * KC, :],
                        d_ws.ap()[:, soff_d:soff_d + ns_w])
                soff_d += ns_w

            # ---- internal waves -------------------------------------------
            soff = 0
            for wi in range(1, nwaves):
                ns, per_core, rep = wave_slots[wi - 1]
                p0, p1 = waves[wi]
                n = p1 - p0
                nch = n * K
                # the heap column order makes the children of all but the
                # last parent one contiguous block in h_bf/c_sl: read it
                # directly; only the tail parent's cols go through staging
                runs = wave_runs[wi - 1]
                if runs and runs[0][0] == 0:
                    src0 = runs[0][1]
                    main_np = runs[0][2] // K
                else:
                    src0, main_np = 0, 0
                main_ln = main_np * K
                tail_ln = nch - main_ln
                hct = wk.tile([P, KC, K * NMAX], BF16, tag="hct")
                cct = wk.tile([P, K * NMAX], F32, tag="cct")
                for (dst, src, ln) in runs:
                    lo = max(dst, main_ln)
                    hi = dst + ln
                    if hi <= lo:
                        continue
                    o = lo - dst
                    nc.vector.tensor_copy(
                        cct[:, lo - main_ln:hi - main_ln],
                        c_sl[:, src + o:src + o + (hi - lo)])
                    nc.gpsimd.tensor_copy(
                        hct[:, :, lo - main_ln:hi - main_ln],
                        h_bf[:, :, src + o:src + o + (hi - lo)])
                # hsum over child cols (bf16 in, bf16 out); one 4D reduce
                # per source covers all feature chunks
                hsum_b = wk.tile([P, KC, NMAX], BF16, tag="hsumb")
                with nc.allow_low_precision(reason="4-term bf16 child sum"):
                    if main_np:
                        nc.vector.tensor_reduce(
                            hsum_b[:, :, :main_np],
                            h_bf[:, :, src0:src0 + main_ln].rearrange(
                                "p k (n c) -> p k n c", c=K),
                            axis=mybir.AxisListType.X,
                            op=mybir.AluOpType.add)
                    if tail_ln:
                        nc.vector.tensor_reduce(
                            hsum_b[:, :, main_np:n],
                            hct[:, :, :tail_ln].rearrange(
                                "p k (n c) -> p k n c", c=K),
                            axis=mybir.AxisListType.X,
                            op=mybir.AluOpType.add)

                all_id = (ns == 0)
                # replicated single-rel wave: every mask is all-ones, feed
                # hsum straight to the matmul (no msel op or edge)
                uniform = rep and ns == 1
                if not all_id:
                    psl = pp.tile([P, MC, PSN], F32, tag="psl")
                    msels = []
                    for s in range(ns):
                        if uniform:
                            msels.append(hsum_b)
                            continue
                        msel = mselp.tile([P, KC, NMAX], BF16,
                                          tag="msel" + str(s))
                        eng = nc.vector if s % 2 == 0 else nc.gpsimd
                        eng.tensor_mul(msel[:, :, :n], hsum_b[:, :, :n],
                                       msk[:, soff + s, :, :n])
                        msels.append(msel)
                    # m-outer so each psum region's accumulation group
                    # (spanning all slots) closes before the next opens
                    for m in range(MC):
                        for s in range(ns):
                            for k in range(KC):
                                nc.tensor.matmul(
                                    psl[:, m, :n],
                                    wrel[:, (soff + s) * MC * KC + m * KC + k, :],
                                    msels[s][:, k, :n],
                                    start=(s == 0 and k == 0),
                                    stop=(s == ns - 1 and k == KC - 1))
                    # scale fp8 weights back (1/WSCALE) on the psum read
                    cb = wk.tile([P, KC, n], BF16, tag="cb" + str(n))
                    nc.scalar.activation(
                        cb[:, :, :n], psl[:, :, :n], ACT.Identity,
                        scale=1.0 / WSCALE)
                    if rep:
                        rhs = cb     # every core computed the full ch_sum
                    else:
                        g1in = dp.tile([P, KC, n], BF16, tag="g1in" + str(n))
                        nc.gpsimd.dma_start(g1in[:], cb[:])
                        g1out = dp.tile([P, KC, n], BF16, tag="g1out" + str(n),
                                        addr_space="Shared")
                        # contributions have disjoint support (masked), so
                        # the bf16 CCE adds are exact
                        nc.gpsimd.collective_compute(
                            "AllReduce", mybir.AluOpType.add,
                            ins=[g1in.opt()], outs=[g1out.opt()],
                            replica_groups=[list(range(NCORES))])
                        chs_b = wk.tile([P, KC, n], BF16, tag="chsb" + str(n))
                        # split the load so the first iou matmuls start at
                        # half the DMA latency
                        nc.gpsimd.dma_start(chs_b[:, :KC // 2, :],
                                            g1out[:, :KC // 2, :])
                        nc.gpsimd.dma_start(chs_b[:, KC // 2:, :],
                                            g1out[:, KC // 2:, :])
                        rhs = chs_b
                else:
                    rhs = hsum_b

                # fh matmuls first: they read h_bf/hct directly, so PE runs
                # them while the AllReduce is in flight; xf joins via eye mm
                psf = pg.tile([P, K * NMAX], F32, tag="psf")
                off = woff[wi - 1] * K
                if main_ln:
                    for k in range(KC):
                        nc.tensor.matmul(
                            psf[:, :main_ln], fhs[:, k, :],
                            h_bf[:, k, src0:src0 + main_ln],
                            start=(k == 0), stop=False)
                    nc.tensor.matmul(psf[:, :main_ln], eye[:],
                                     xfb_all[:, off:off + main_ln],
                                     start=False, stop=True)
                if tail_ln:
                    for k in range(KC):
                        nc.tensor.matmul(
                            psf[:, main_ln:nch], fhs[:, k, :],
                            hct[:, k, :tail_ln],
                            start=(k == 0), stop=False)
                    nc.tensor.matmul(psf[:, main_ln:nch], eye[:],
                                     xfb_all[:, off + main_ln:off + nch],
                                     start=False, stop=True)
                # iou matmuls (column-sharded); xi joins via eye matmul
                psi = pg.tile([P, 3, PSN], F32, tag="ps3")
                for g in range(3):
                    for k in range(KC):
                        nc.tensor.matmul(
                            psi[:, g, :n], iouhs[:, k * 3 + g, :],
                            rhs[:, k, :n],
                            start=(k == 0), stop=False)
                    nc.tensor.matmul(psi[:, g, :n], eye[:],
                                     xi_bf[:, g, p0:p0 + n],
                                     start=False, stop=True)
                gates(p0, n, psi, psf[:, :nch],
                      cc_main=(c_sl[:, src0:src0 + main_ln]
                               if main_ln else None),
                      cc_tail=(cct[:, :tail_ln] if tail_ln else None),
                      main_ln=main_ln, nch=nch)
                nc.sync.dma_start(d_hout.ap()[:, p0:p0 + n],
                                  h_sl[:, p0:p0 + n])
                if wi < nwaves - 1:
                    publish_h(p0, n)
                soff += ns

    in_maps = []
    for c in range(NCORES):
        in_maps.append({
            "wslots": wslots[c], "masks": masks_x[c],
            "xt": xt_h, "iouxstat": iouxstat[c], "iouhstat": iouhstat[c],
            "fxstat": fxstat[c], "fhstat": fhstat[c], "eye": eye_h,
            "b_xi": b_xi[c], "b_iou": b_iou[c], "b_xf": b_xf[c],
            "b_fh": b_fh[c],
        })
    _split_multi_waits(nc)
    return nc, in_maps, col_of, N, MEM


def kernel(**inputs):
    nc, in_maps, col_of, N, MEM = _build(inputs)
    kernel._nc = nc
    kernel._in_maps = in_maps
    res = run_bass_kernel_spmd(nc, in_maps, list(range(NCORES)))
    hT = np.concatenate(
        [res.results[c]["hout"].astype(np.float32) for c in range(NCORES)], 0)
    out = np.empty((N, MEM), np.float32)
    for node in range(N):
        out[node] = hT[:, col_of[node]]
    return out
